# revision 6
# baseline (speedup 1.0000x reference)
"""Trainium2 Bass kernel for nn_LowRankAttention (8 NeuronCores, SPMD).

Sharding: core c = 2*b + s handles batch b = c//2 and sequence-half s = c%2.
Each core independently computes the full attention pipeline for its batch
over its 258 query rows (256 owned rows + halo/dummy rows) and all 512
key/value rows; there is no cross-core communication.

All matmuls run in float32r (TF32-like fast fp32 mode, full PE rate for
moving dims >= 256), with fp32 PSUM accumulation.
"""

import numpy as np

B, L, C = 4, 4096, 1024
H, D = 8, 64
HALF = 32
R = 512            # low-rank rows (L ** 0.75)
NQ = 258           # per-core query rows: 256 owned + halo + dummy
EPS = 1e-6
LAM_INIT = 0.8
ROPE_BASE = 10000.0
SCALE = D ** -0.5
NCORES = 8
LSH = L // 2       # output rows per core (2048)

_CACHE = {}


def _rope_tables():
    pos = np.arange(H, dtype=np.float64)
    freqs = 1.0 / (ROPE_BASE ** (np.arange(HALF, dtype=np.float64) / HALF))
    ang = pos[:, None] * freqs                       # (H, 32)
    return np.cos(ang), np.sin(ang)


def _rt_tables():
    """RT_h[j, i] such that rope(y)_i = sum_j y_j * RT_h[j, i] (per head h).

    Stacked over heads -> [64, H*64]; duplicated over both 64-partition
    halves -> [128, H*64] so path-0/path-1 matmuls can use base partition
    0/64 with identical content.  rtq additionally folds the attention
    scale (1/sqrt(d)).
    """
    cos, sin = _rope_tables()
    rt = np.zeros((H, D, D), dtype=np.float64)
    for h in range(H):
        for i in range(HALF):
            rt[h, i, i] = cos[h, i]
            rt[h, i + HALF, i] = -sin[h, i]
            rt[h, i, i + HALF] = sin[h, i]
            rt[h, i + HALF, i + HALF] = cos[h, i]
    flat = rt.transpose(1, 0, 2).reshape(D, H * D)   # [j, h*64+i]
    rtq = (flat * SCALE).astype(np.float32)
    rtk = flat.astype(np.float32)
    return (np.concatenate([rtq, rtq], axis=0),
            np.concatenate([rtk, rtk], axis=0))      # [128, 512]


def _upsample_tables(s):
    """Chunk-aligned banded interp matrices ub_a, ub_b [128, 2048] and the
    per-out-chunk segment list. LR local row i corresponds to global low-rank
    row A_s + i with A_s = 256*s - 1."""
    A = 256 * s - 1
    U = np.zeros((NQ + 128, 16 * 128), dtype=np.float64)
    for j in range(LSH):
        jglob = LSH * s + j
        coords = min(max((jglob + 0.5) / 8.0 - 0.5, 0.0), float(R - 1))
        lo = int(np.floor(coords))
        hi = min(lo + 1, R - 1)
        w = coords - lo
        U[lo - A, j] += 1.0 - w
        U[hi - A, j] += w
    ub_a = np.zeros((128, 16 * 128), dtype=np.float32)
    ub_b = np.zeros((2, 16 * 128), dtype=np.float32)
    segs = []
    for oc in range(16):
        cols = slice(128 * oc, 128 * oc + 128)
        p0 = (16 * oc) // 128
        ub_a[:, cols] = U[128 * p0:128 * p0 + 128, cols]
        crossing = (16 * oc + 18) > 128 * (p0 + 1)
        if crossing:
            ub_b[:, cols] = U[128 * (p0 + 1):128 * (p0 + 1) + 2, cols]
        segs.append((p0, crossing))
    return ub_a, ub_b, segs


def _build_nc(segs):
    import concourse.bacc as bacc
    import concourse.tile as tile
    from concourse import mybir

    F32 = mybir.dt.float32
    F32R = mybir.dt.float32r
    AF = mybir.ActivationFunctionType
    ALU = mybir.AluOpType

    nc = bacc.Bacc("TRN2", target_bir_lowering=False, debug=False)
    dp = nc.declare_dram_parameter
    i_xlo = dp("xlo", [R, C], F32, isOutput=False)
    i_xhi = dp("xhi", [R, C], F32, isOutput=False)
    i_xqlo = dp("xqlo", [NQ, C], F32, isOutput=False)
    i_xqhi = dp("xqhi", [NQ, C], F32, isOutput=False)
    i_wq = dp("wqT", [C, C], F32R, isOutput=False)
    i_wk = dp("wkT", [C, C], F32R, isOutput=False)
    i_wv = dp("wvT", [C, C], F32R, isOutput=False)
    i_wo = dp("woT", [C, C], F32R, isOutput=False)
    i_rtq = dp("rtq", [128, H * D], F32, isOutput=False)
    i_rtk = dp("rtk", [128, H * D], F32, isOutput=False)
    i_qw = dp("qw2", [128, 1], F32, isOutput=False)
    i_kw = dp("kw2", [128, 1], F32, isOutput=False)
    i_hw = dp("hw", [128, 1], F32, isOutput=False)
    i_lam = dp("lamvec", [1, 4 * D], F32, isOutput=False)
    i_m2 = dp("m2", [128, 2], F32R, isOutput=False)
    i_e2 = dp("e2", [2, 128], F32R, isOutput=False)
    i_onesr = dp("onesr", [1, 128], F32R, isOutput=False)
    i_ones128 = dp("ones128", [128, 1], F32R, isOutput=False)
    i_ident = dp("ident", [128, 128], F32, isOutput=False)
    i_uba = dp("ub_a", [128, 16 * 128], F32R, isOutput=False)
    i_ubb = dp("ub_b", [2, 16 * 128], F32R, isOutput=False)
    o_fo = dp("fo", [LSH, C], F32, isOutput=True)
    o_lo = dp("lo", [256, C], F32, isOutput=True)

    with nc.allow_low_precision(reason="float32r matmul pipeline"), \
            tile.TileContext(nc) as tc:
        from contextlib import ExitStack
        es = ExitStack()
        cpool = es.enter_context(tc.tile_pool(name="consts", bufs=1))
        ps = es.enter_context(tc.tile_pool(name="ps", bufs=8, space="PSUM"))
        projp = es.enter_context(tc.tile_pool(name="projp", bufs=1))

        # ---- constants ----
        qw_t = cpool.tile([128, 1], F32)
        kw_t = cpool.tile([128, 1], F32)
        hw_t = cpool.tile([128, 1], F32)
        hw2_t = cpool.tile([128, 1], F32)
        lam_t = cpool.tile([1, 4 * D], F32)
        m2_t = cpool.tile([128, 2], F32R)
        e2_t = cpool.tile([2, 128], F32R)
        onesr_t = cpool.tile([1, 128], F32R)
        ones128_t = cpool.tile([128, 1], F32R)
        ident_t = cpool.tile([128, 128], F32)
        for t, srcp in [(qw_t, i_qw),
                        (kw_t, i_kw), (hw_t, i_hw), (lam_t, i_lam),
                        (m2_t, i_m2), (e2_t, i_e2), (onesr_t, i_onesr),
                        (ones128_t, i_ones128), (ident_t, i_ident)]:
            nc.sync.dma_start(t[:], srcp[:])
        mq_t = cpool.tile([128, H * D], F32R)
        mk_t = cpool.tile([128, H * D], F32R)
        nc.scalar.mul(hw2_t[:], hw_t[:], 1.0 - LAM_INIT)
        eps2_t = cpool.tile([2, 1], F32)
        nc.vector.memset(eps2_t[:], EPS)

        # ---- xd / xq build + transposes ----
        xdT = projp.tile([128, 8, R], F32R)     # [in_ch_chunk, kc, row]
        xqT = projp.tile([128, 8, NQ], F32R)
        with tc.tile_pool(name="xio", bufs=4) as xio, \
                tc.tile_pool(name="xs", bufs=7) as xsp:
            rtq_t = xio.tile([128, H * D], F32, tag="rt")
            rtk_t = xio.tile([128, H * D], F32, tag="rt")
            nc.sync.dma_start(rtq_t[:], i_rtq[:])
            nc.sync.dma_start(rtk_t[:], i_rtk[:])
            nc.vector.tensor_scalar_mul(mq_t[:], rtq_t[:], qw_t[:])
            nc.vector.tensor_scalar_mul(mk_t[:], rtk_t[:], kw_t[:])
            xs = []
            for rc in range(4):
                a = xio.tile([128, C], F32, tag="xin")
                b2 = xio.tile([128, C], F32, tag="xin")
                nc.sync.dma_start(a[:], i_xlo[128 * rc:128 * rc + 128, :])
                nc.sync.dma_start(b2[:], i_xhi[128 * rc:128 * rc + 128, :])
                t = xsp.tile([128, C], F32)
                nc.vector.tensor_tensor(t[:], a[:], b2[:], op=ALU.add)
                xs.append(t)
            xqs = []
            qrows = (128, 128, 2)
            for rc in range(3):
                rows = qrows[rc]
                a = xio.tile([128, C], F32, tag="xin")
                b2 = xio.tile([128, C], F32, tag="xin")
                nc.sync.dma_start(a[0:rows, :],
                                  i_xqlo[128 * rc:128 * rc + rows, :])
                nc.sync.dma_start(b2[0:rows, :],
                                  i_xqhi[128 * rc:128 * rc + rows, :])
                t = xsp.tile([128, C], F32)
                nc.vector.tensor_tensor(t[0:rows, :], a[0:rows, :],
                                        b2[0:rows, :], op=ALU.add)
                xqs.append(t)
            for kc in range(8):
                for rc in range(4):
                    pst = ps.tile([128, 128], F32, tag="ps")
                    nc.tensor.transpose(pst[:], xs[rc][:, 128 * kc:128 * kc + 128],
                                        ident_t[:])
                    nc.scalar.activation(xdT[:, kc, 128 * rc:128 * rc + 128],
                                         pst[:], AF.Copy, scale=0.5)
                for rc in range(3):
                    rows = qrows[rc]
                    pst = ps.tile([128, 128], F32, tag="ps")
                    nc.tensor.transpose(pst[:, 0:rows],
                                        xqs[rc][0:rows, 128 * kc:128 * kc + 128],
                                        ident_t[0:rows, 0:rows])
                    nc.scalar.activation(xqT[:, kc, 128 * rc:128 * rc + rows],
                                         pst[:, 0:rows], AF.Copy, scale=0.5)

        # ---- Q / K projections with fused rmsnorm + rope ----
        big = es.enter_context(tc.tile_pool(name="big", bufs=1))
        wpool = es.enter_context(tc.tile_pool(name="w", bufs=9))
        qkp = es.enter_context(tc.tile_pool(name="qk", bufs=2))
        attp = es.enter_context(tc.tile_pool(name="att", bufs=2))
        tmp = es.enter_context(tc.tile_pool(name="tmp", bufs=2))
        small = es.enter_context(tc.tile_pool(name="small", bufs=2))
        obuf = es.enter_context(tc.tile_pool(name="ob", bufs=2))
        # lam = exp(sum(lq1*lk1)) - exp(sum(lq2*lk2)) + 0.8 ; store -lam
        l1 = cpool.tile([1, D], F32)
        l2 = cpool.tile([1, D], F32)
        nc.vector.tensor_tensor(l1[:], lam_t[0:1, 0:D], lam_t[0:1, D:2 * D],
                                op=ALU.mult)
        nc.vector.tensor_tensor(l2[:], lam_t[0:1, 2 * D:3 * D],
                                lam_t[0:1, 3 * D:4 * D], op=ALU.mult)
        s1 = cpool.tile([1, 1], F32)
        s2 = cpool.tile([1, 1], F32)
        nc.vector.reduce_sum(s1[:], l1[:], axis=mybir.AxisListType.X)
        nc.vector.reduce_sum(s2[:], l2[:], axis=mybir.AxisListType.X)
        e1s = cpool.tile([1, 1], F32)
        e2s = cpool.tile([1, 1], F32)
        nc.scalar.activation(e1s[:], s1[:], AF.Exp)
        nc.scalar.activation(e2s[:], s2[:], AF.Exp)
        dls = cpool.tile([1, 1], F32)
        nc.vector.tensor_tensor(dls[:], e1s[:], e2s[:], op=ALU.subtract)
        lamneg_t = cpool.tile([1, 1], F32)
        nc.scalar.activation(lamneg_t[:], dls[:], AF.Copy, scale=-1.0,
                             bias=-LAM_INIT)
        qf = big.tile([128, 8, NQ], F32R)       # [2 paths x 64 d, head, qrow]
        kf = big.tile([128, 8, R], F32R)

        def qk_proj(w_sb, xT, n, out_f, m_t):
            for h in range(8):
                psq = ps.tile([128, n], F32, tag="ps")
                for kc in range(8):
                    nc.tensor.matmul(psq[:], w_sb[kc][:, 128 * h:128 * h + 128],
                                     xT[:, kc, :], start=(kc == 0),
                                     stop=(kc == 7))
                rawf = qkp.tile([128, 512], F32R, tag="raw")
                raw = rawf[:, 0:n]
                nc.vector.tensor_copy(raw, psq[:])
                sqf = qkp.tile([128, 512], F32R, tag="sq")
                sq = sqf[:, 0:n]
                nc.vector.tensor_tensor(sq, raw.bitcast(F32),
                                        raw.bitcast(F32), op=ALU.mult)
                pss = ps.tile([2, n], F32, tag="ps")
                nc.tensor.matmul(pss[:], m2_t[:], sq, start=True, stop=True)
                tqf = small.tile([2, 512], F32, tag="tq")
                tq = tqf[:, 0:n]
                nc.scalar.activation(tq, pss[:], AF.Sqrt,
                                     bias=eps2_t[:], scale=1.0 / D)
                rqf = small.tile([2, 512], F32R, tag="rq")
                rq = rqf[:, 0:n]
                nc.vector.reciprocal(rq, tq)
                psr = ps.tile([128, n], F32, tag="ps")
                nc.tensor.matmul(psr[:], e2_t[:], rq, start=True, stop=True)
                repf = qkp.tile([128, 512], F32, tag="rep")
                rep = repf[:, 0:n]
                nc.vector.tensor_copy(rep, psr[:])
                for p in range(2):
                    pp = ps.tile([64, n], F32, tag="ps")
                    sl = slice(64 * p, 64 * p + 64)
                    nc.tensor.matmul(pp[:], m_t[sl, 64 * h:64 * h + 64],
                                     raw[sl, :], start=True, stop=True)
                    nc.vector.tensor_tensor(out_f[sl, h, :], pp[:],
                                            rep[sl, :], op=ALU.mult)

        wq_sb = []
        for kc in range(8):
            t = wpool.tile([128, C], F32R, tag="w")
            nc.sync.dma_start(t[:], i_wq[128 * kc:128 * kc + 128, :])
            wq_sb.append(t)
        qk_proj(wq_sb, xqT, NQ, qf, mq_t)
        wk_sb = []
        for kc in range(8):
            t = wpool.tile([128, C], F32R, tag="w")
            nc.sync.dma_start(t[:], i_wk[128 * kc:128 * kc + 128, :])
            wk_sb.append(t)
        qk_proj(wk_sb, xdT, R, kf, mk_t)

        # ---- V projection (row-major) ----
        v_t = big.tile([128, 4, C], F32R)       # [krow_chunk, rc, vch]
        wv_sb = []
        for kc in range(8):
            t = wpool.tile([128, C], F32R, tag="w")
            nc.sync.dma_start(t[:], i_wv[128 * kc:128 * kc + 128, :])
            wv_sb.append(t)
        for rc in range(4):
            for n2 in range(2):
                psv = ps.tile([128, 512], F32, tag="ps")
                for kc in range(8):
                    nc.tensor.matmul(psv[:], xdT[:, kc, 128 * rc:128 * rc + 128],
                                     wv_sb[kc][:, 512 * n2:512 * n2 + 512],
                                     start=(kc == 0), stop=(kc == 7))
                nc.scalar.copy(v_t[:, rc, 512 * n2:512 * n2 + 512], psv[:])

        # ---- attention + head norm, per head ----
        dn = big.tile([128, 8, NQ], F32R)       # [in_ch_chunk, h, qrow]
        for h in range(8):
            ex = [attp.tile([128, 4, NQ], F32R, tag="e%d" % p, name="ex%d" % p)
                  for p in range(2)]
            for p in range(2):
                sl = slice(64 * p, 64 * p + 64)
                for rc in range(4):
                    pssc = ps.tile([128, NQ], F32, tag="ps")
                    nc.tensor.matmul(pssc[:], kf[sl, h, 128 * rc:128 * rc + 128],
                                     qf[sl, h, :], start=True, stop=True)
                    nc.scalar.activation(ex[p][:, rc, :], pssc[:], AF.Exp)
            reps = []
            for p in range(2):
                pssum = ps.tile([1, NQ], F32, tag="ps")
                for rc in range(4):
                    nc.tensor.matmul(pssum[:], ones128_t[:], ex[p][:, rc, :],
                                     start=(rc == 0), stop=(rc == 3))
                rs = small.tile([1, NQ], F32R, tag="rs")
                nc.vector.reciprocal(rs[:], pssum[:])
                if p == 1:
                    rs2 = small.tile([1, NQ], F32R, tag="rs")
                    nc.vector.tensor_scalar_mul(rs2[:], rs[:].bitcast(F32),
                                                lamneg_t[:])
                    rs = rs2
                psrep = ps.tile([128, NQ], F32, tag="ps")
                nc.tensor.matmul(psrep[:], onesr_t[:], rs[:], start=True,
                                 stop=True)
                rep = tmp.tile([128, NQ], F32, tag="rep")
                nc.vector.tensor_copy(rep[:], psrep[:])
                reps.append(rep)
            parts = []
            for p in range(2):
                pspv = ps.tile([128, NQ], F32, tag="ps")
                for rc in range(4):
                    nc.tensor.matmul(pspv[:], v_t[:, rc, 128 * h:128 * h + 128],
                                     ex[p][:, rc, :], start=(rc == 0),
                                     stop=(rc == 3))
                t = tmp.tile([128, NQ], F32, tag="pvt")
                nc.vector.tensor_tensor(t[:], pspv[:], reps[p][:], op=ALU.mult)
                parts.append(t)
            draw = tmp.tile([128, NQ], F32, tag="draw")
            nc.vector.tensor_tensor(draw[:], parts[0][:], parts[1][:],
                                    op=ALU.add)
            # head rmsnorm over 128 channels
            sqd = tmp.tile([128, NQ], F32R, tag="sqd")
            nc.vector.tensor_tensor(sqd[:], draw[:], draw[:], op=ALU.mult)
            pshs = ps.tile([1, NQ], F32, tag="ps")
            nc.tensor.matmul(pshs[:], ones128_t[:], sqd[:], start=True,
                             stop=True)
            th = small.tile([1, NQ], F32, tag="th")
            nc.scalar.activation(th[:], pshs[:], AF.Sqrt,
                                 bias=eps2_t[0:1, :], scale=1.0 / 128)
            rh = small.tile([1, NQ], F32R, tag="rh")
            nc.vector.reciprocal(rh[:], th[:])
            pshr = ps.tile([128, NQ], F32, tag="ps")
            nc.tensor.matmul(pshr[:], onesr_t[:], rh[:], start=True, stop=True)
            dnt = tmp.tile([128, NQ], F32, tag="dnt")
            nc.vector.tensor_tensor(dnt[:], pshr[:], draw[:], op=ALU.mult)
            nc.vector.tensor_scalar_mul(dn[:, h, :], dnt[:], hw2_t[:])

        # ---- Wo + silu ----
        lr = big.tile([128, 3, C], F32R)
        wo_sb = []
        for kc in range(8):
            t = wpool.tile([128, C], F32R, tag="w")
            nc.sync.dma_start(t[:], i_wo[128 * kc:128 * kc + 128, :])
            wo_sb.append(t)
        mrows = (128, 128, 2)
        for mc in range(3):
            rows = mrows[mc]
            for n2 in range(2):
                pso = ps.tile([128, 512], F32, tag="ps")
                for kc in range(8):
                    nc.tensor.matmul(pso[0:rows, :],
                                     dn[:, kc, 128 * mc:128 * mc + rows],
                                     wo_sb[kc][:, 512 * n2:512 * n2 + 512],
                                     start=(kc == 0), stop=(kc == 7))
                nc.scalar.activation(lr[0:rows, mc, 512 * n2:512 * n2 + 512],
                                     pso[0:rows, :], AF.Silu)

        # ---- upsample (banded interp matmuls) + outputs ----
        ub_t = {}
        ubb_t = cpool.tile([2, 16 * 128], F32R, name="ubbt")
        nc.sync.dma_start(ubb_t[:], i_ubb[:])
        for half in range(2):
            ta = wpool.tile([128, C], F32R, tag="w", name="uba%d" % half)
            nc.sync.dma_start(ta[:], i_uba[:, C * half:C * half + C])
            ub_t[half] = ta
        for oc in range(16):
            p0, crossing = segs[oc]
            uba_t = ub_t[oc // 8]
            col = 128 * (oc % 8)
            colg = 128 * oc
            ob = obuf.tile([128, C], F32, tag="ob")
            for n2 in range(2):
                psu = ps.tile([128, 512], F32, tag="ps")
                nc.tensor.matmul(psu[:], uba_t[:, col:col + 128],
                                 lr[:, p0, 512 * n2:512 * n2 + 512],
                                 start=True, stop=not crossing)
                if crossing:
                    nc.tensor.matmul(psu[:], ubb_t[0:2, colg:colg + 128],
                                     lr[0:2, p0 + 1, 512 * n2:512 * n2 + 512],
                                     start=False, stop=True)
                nc.vector.tensor_copy(ob[:, 512 * n2:512 * n2 + 512], psu[:])
            nc.sync.dma_start(o_fo[128 * oc:128 * oc + 128, :], ob[:])
        # lowrank shard: local rows 1..256  ->  global [256*s, 256*s+256)
        nc.sync.dma_start(o_lo[0:127, :], lr[1:128, 0, :].bitcast(F32))
        nc.sync.dma_start(o_lo[127:255, :], lr[0:128, 1, :].bitcast(F32))
        nc.sync.dma_start(o_lo[255:256, :], lr[0:1, 2, :].bitcast(F32))
        es.close()
    nc.finalize()
    return nc


def _get_built():
    if "nc" not in _CACHE:
        _, _, segs = _upsample_tables(0)
        segs1 = _upsample_tables(1)[2]
        assert segs == segs1, "segment structure must be core-uniform"
        _CACHE["nc"] = _build_nc(segs)
    return _CACHE["nc"]


def _in_maps(x, Wq, Wk, Wv, Wo, q_norm_w, k_norm_w, head_norm_w,
             lambda_q1, lambda_k1, lambda_q2, lambda_k2):
    f = np.float32
    wqT = np.ascontiguousarray(Wq.T, dtype=f)
    wkT = np.ascontiguousarray(Wk.T, dtype=f)
    wvT = np.ascontiguousarray(Wv.T, dtype=f)
    woT = np.ascontiguousarray(Wo.T, dtype=f)
    rtq, rtk = _rt_tables()
    qw2 = np.tile(np.asarray(q_norm_w, f), 2).reshape(128, 1)
    kw2 = np.tile(np.asarray(k_norm_w, f), 2).reshape(128, 1)
    hw = np.asarray(head_norm_w, f).reshape(128, 1)
    lamvec = np.concatenate([np.asarray(a, f) for a in
                             (lambda_q1, lambda_k1, lambda_q2, lambda_k2)]
                            ).reshape(1, 4 * D)
    m2 = np.zeros((128, 2), f)
    m2[:D, 0] = 1.0
    m2[D:, 1] = 1.0
    e2 = np.zeros((2, 128), f)
    e2[0, :D] = 1.0
    e2[1, D:] = 1.0
    onesr = np.ones((1, 128), f)
    ones128 = np.ones((128, 1), f)
    ident = np.eye(128, dtype=f)
    shared = dict(wqT=wqT, wkT=wkT, wvT=wvT, woT=woT, rtq=rtq, rtk=rtk,
                  qw2=qw2, kw2=kw2, hw=hw, lamvec=lamvec, m2=m2, e2=e2,
                  onesr=onesr, ones128=ones128, ident=ident)
    ubs = [_upsample_tables(s)[:2] for s in range(2)]
    maps = []
    for c in range(NCORES):
        b, s = c // 2, c % 2
        xlo = np.ascontiguousarray(x[b, 3::8, :], dtype=f)
        xhi = np.ascontiguousarray(x[b, 4::8, :], dtype=f)
        A = 256 * s - 1
        xqlo = np.zeros((NQ, C), f)
        xqhi = np.zeros((NQ, C), f)
        g0, g1 = max(0, A), min(R, A + NQ)
        xqlo[g0 - A:g1 - A] = xlo[g0:g1]
        xqhi[g0 - A:g1 - A] = xhi[g0:g1]
        m = dict(shared)
        m.update(xlo=xlo, xhi=xhi, xqlo=xqlo, xqhi=xqhi,
                 ub_a=ubs[s][0], ub_b=ubs[s][1])
        maps.append(m)
    return maps


def kernel(**inputs):
    from concourse import bass_utils
    nc = _get_built()
    maps = _in_maps(**inputs)
    res = bass_utils.run_bass_kernel_spmd(nc, maps, core_ids=list(range(NCORES)))
    full = np.empty((B, L, C), np.float32)
    low = np.empty((B, R, C), np.float32)
    for c in range(NCORES):
        b, s = c // 2, c % 2
        full[b, LSH * s:LSH * s + LSH, :] = res.results[c]["fo"]
        low[b, 256 * s:256 * s + 256, :] = res.results[c]["lo"]
    return full, low


# revision 8
# speedup vs baseline: 1.2008x; 1.2008x over previous
"""Trainium2 Bass kernel for nn_LowRankAttention (8 NeuronCores, SPMD).

Sharding: core c = 2*b + s handles batch b = c//2 and sequence-half s = c%2.
Each core independently computes the full attention pipeline for its batch
over its 258 query rows (256 owned rows + halo/dummy rows) and all 512
key/value rows; there is no cross-core communication.

All matmuls run in float32r (TF32-like fast fp32 mode, full PE rate for
moving dims >= 256), with fp32 PSUM accumulation.
"""

import numpy as np

B, L, C = 4, 4096, 1024
H, D = 8, 64
HALF = 32
R = 512            # low-rank rows (L ** 0.75)
NQ = 258           # per-core query rows: 256 owned + halo + dummy
EPS = 1e-6
LAM_INIT = 0.8
ROPE_BASE = 10000.0
SCALE = D ** -0.5
NCORES = 8
LSH = L // 2       # output rows per core (2048)

_CACHE = {}


def _rope_tables():
    pos = np.arange(H, dtype=np.float64)
    freqs = 1.0 / (ROPE_BASE ** (np.arange(HALF, dtype=np.float64) / HALF))
    ang = pos[:, None] * freqs                       # (H, 32)
    return np.cos(ang), np.sin(ang)


def _rt_tables():
    """RT_h[j, i] such that rope(y)_i = sum_j y_j * RT_h[j, i] (per head h).

    Stacked over heads -> [64, H*64]; duplicated over both 64-partition
    halves -> [128, H*64] so path-0/path-1 matmuls can use base partition
    0/64 with identical content.  rtq additionally folds the attention
    scale (1/sqrt(d)).
    """
    cos, sin = _rope_tables()
    rt = np.zeros((H, D, D), dtype=np.float64)
    for h in range(H):
        for i in range(HALF):
            rt[h, i, i] = cos[h, i]
            rt[h, i + HALF, i] = -sin[h, i]
            rt[h, i, i + HALF] = sin[h, i]
            rt[h, i + HALF, i + HALF] = cos[h, i]
    flat = rt.transpose(1, 0, 2).reshape(D, H * D)   # [j, h*64+i]
    rtq = (flat * SCALE).astype(np.float32)
    rtk = flat.astype(np.float32)
    return (np.concatenate([rtq, rtq], axis=0),
            np.concatenate([rtk, rtk], axis=0))      # [128, 512]


def _upsample_tables(s):
    """Chunk-aligned banded interp matrices ub_a, ub_b [128, 2048] and the
    per-out-chunk segment list. LR local row i corresponds to global low-rank
    row A_s + i with A_s = 256*s - 1."""
    A = 256 * s - 1
    U = np.zeros((NQ + 128, 16 * 128), dtype=np.float64)
    for j in range(LSH):
        jglob = LSH * s + j
        coords = min(max((jglob + 0.5) / 8.0 - 0.5, 0.0), float(R - 1))
        lo = int(np.floor(coords))
        hi = min(lo + 1, R - 1)
        w = coords - lo
        U[lo - A, j] += 1.0 - w
        U[hi - A, j] += w
    ub_a = np.zeros((128, 16 * 128), dtype=np.float32)
    ub_b = np.zeros((2, 16 * 128), dtype=np.float32)
    segs = []
    for oc in range(16):
        cols = slice(128 * oc, 128 * oc + 128)
        p0 = (16 * oc) // 128
        ub_a[:, cols] = U[128 * p0:128 * p0 + 128, cols]
        crossing = (16 * oc + 18) > 128 * (p0 + 1)
        if crossing:
            ub_b[:, cols] = U[128 * (p0 + 1):128 * (p0 + 1) + 2, cols]
        segs.append((p0, crossing))
    return ub_a, ub_b, segs


def _build_nc(segs):
    import concourse.bacc as bacc
    import concourse.tile as tile
    from concourse import mybir

    F32 = mybir.dt.float32
    F32R = mybir.dt.float32r
    AF = mybir.ActivationFunctionType
    ALU = mybir.AluOpType

    nc = bacc.Bacc("TRN2", target_bir_lowering=False, debug=False)
    dp = nc.declare_dram_parameter
    i_xlo = dp("xlo", [R, C], F32, isOutput=False)
    i_xhi = dp("xhi", [R, C], F32, isOutput=False)
    i_xqlo = dp("xqlo", [NQ, C], F32, isOutput=False)
    i_xqhi = dp("xqhi", [NQ, C], F32, isOutput=False)
    i_wq = dp("wqT", [C, C], F32R, isOutput=False)
    i_wk = dp("wkT", [C, C], F32R, isOutput=False)
    i_wv = dp("wvT", [C, C], F32R, isOutput=False)
    i_wo = dp("woT", [C, C], F32R, isOutput=False)
    i_rtq = dp("rtq", [128, H * D], F32, isOutput=False)
    i_rtk = dp("rtk", [128, H * D], F32, isOutput=False)
    i_qw = dp("qw2", [128, 1], F32, isOutput=False)
    i_kw = dp("kw2", [128, 1], F32, isOutput=False)
    i_hw = dp("hw", [128, 1], F32, isOutput=False)
    i_lam = dp("lamvec", [1, 4 * D], F32, isOutput=False)
    i_m2 = dp("m2", [128, 33], F32R, isOutput=False)
    i_e2 = dp("e2", [2, 128], F32R, isOutput=False)
    i_onesr = dp("onesr", [1, 128], F32R, isOutput=False)
    i_ones128 = dp("ones128", [128, 1], F32R, isOutput=False)
    i_ident = dp("ident", [128, 128], F32, isOutput=False)
    i_uba = dp("ub_a", [128, 16 * 128], F32R, isOutput=False)
    i_ubb = dp("ub_b", [2, 16 * 128], F32R, isOutput=False)
    o_fo = dp("fo", [LSH, C], F32, isOutput=True)
    o_lo = dp("lo", [256, C], F32, isOutput=True)

    with nc.allow_low_precision(reason="float32r matmul pipeline"), \
            tile.TileContext(nc) as tc:
        from contextlib import ExitStack
        es = ExitStack()
        cpool = es.enter_context(tc.tile_pool(name="consts", bufs=1))
        ps = es.enter_context(tc.tile_pool(name="ps", bufs=8, space="PSUM"))
        projp = es.enter_context(tc.tile_pool(name="projp", bufs=1))

        # ---- constants ----
        qw_t = cpool.tile([128, 1], F32)
        kw_t = cpool.tile([128, 1], F32)
        hw_t = cpool.tile([128, 1], F32)
        hw2_t = cpool.tile([128, 1], F32)
        lam_t = cpool.tile([1, 4 * D], F32)
        m2_t = cpool.tile([128, 33], F32R)
        e2_t = cpool.tile([2, 128], F32R)
        onesr_t = cpool.tile([1, 128], F32R)
        ones128_t = cpool.tile([128, 1], F32R)
        ident_t = cpool.tile([128, 128], F32)
        for t, srcp in [(qw_t, i_qw),
                        (kw_t, i_kw), (hw_t, i_hw), (lam_t, i_lam),
                        (m2_t, i_m2), (e2_t, i_e2), (onesr_t, i_onesr),
                        (ones128_t, i_ones128), (ident_t, i_ident)]:
            nc.sync.dma_start(t[:], srcp[:])
        mq_t = cpool.tile([128, H * D], F32R)
        mk_t = cpool.tile([128, H * D], F32R)
        nc.scalar.mul(hw2_t[:], hw_t[:], 1.0 - LAM_INIT)
        eps2_t = cpool.tile([2, 1], F32)
        nc.vector.memset(eps2_t[:], EPS)

        # ---- xd / xq build + transposes ----
        xdT = projp.tile([128, 8, R], F32R)     # [in_ch_chunk, kc, row]
        xqT = projp.tile([128, 8, NQ], F32R)
        with tc.tile_pool(name="xio", bufs=4) as xio, \
                tc.tile_pool(name="xs", bufs=7) as xsp:
            rtq_t = xio.tile([128, H * D], F32, tag="rt")
            rtk_t = xio.tile([128, H * D], F32, tag="rt")
            nc.sync.dma_start(rtq_t[:], i_rtq[:])
            nc.sync.dma_start(rtk_t[:], i_rtk[:])
            nc.vector.tensor_scalar_mul(mq_t[:], rtq_t[:], qw_t[:])
            nc.vector.tensor_scalar_mul(mk_t[:], rtk_t[:], kw_t[:])
            xs = []
            for rc in range(4):
                a = xio.tile([128, C], F32, tag="xin")
                b2 = xio.tile([128, C], F32, tag="xin")
                nc.sync.dma_start(a[:], i_xlo[128 * rc:128 * rc + 128, :])
                nc.sync.dma_start(b2[:], i_xhi[128 * rc:128 * rc + 128, :])
                t = xsp.tile([128, C], F32)
                nc.vector.tensor_tensor(t[:], a[:], b2[:], op=ALU.add)
                xs.append(t)
            xqs = []
            qrows = (128, 128, 2)
            for rc in range(3):
                rows = qrows[rc]
                a = xio.tile([128, C], F32, tag="xin")
                b2 = xio.tile([128, C], F32, tag="xin")
                nc.sync.dma_start(a[0:rows, :],
                                  i_xqlo[128 * rc:128 * rc + rows, :])
                nc.sync.dma_start(b2[0:rows, :],
                                  i_xqhi[128 * rc:128 * rc + rows, :])
                t = xsp.tile([128, C], F32)
                nc.vector.tensor_tensor(t[0:rows, :], a[0:rows, :],
                                        b2[0:rows, :], op=ALU.add)
                xqs.append(t)
            for kc in range(8):
                for rc in range(4):
                    pst = ps.tile([128, 128], F32, tag="ps")
                    nc.tensor.transpose(pst[:], xs[rc][:, 128 * kc:128 * kc + 128],
                                        ident_t[:])
                    nc.scalar.activation(xdT[:, kc, 128 * rc:128 * rc + 128],
                                         pst[:], AF.Copy, scale=0.5)
                for rc in range(3):
                    rows = qrows[rc]
                    pst = ps.tile([128, 128], F32, tag="ps")
                    nc.tensor.transpose(pst[:, 0:rows],
                                        xqs[rc][0:rows, 128 * kc:128 * kc + 128],
                                        ident_t[0:rows, 0:rows])
                    nc.scalar.activation(xqT[:, kc, 128 * rc:128 * rc + rows],
                                         pst[:, 0:rows], AF.Copy, scale=0.5)

        # ---- Q / K projections with fused rmsnorm + rope ----
        big = es.enter_context(tc.tile_pool(name="big", bufs=1))
        wpool = es.enter_context(tc.tile_pool(name="w", bufs=9))
        qkp = es.enter_context(tc.tile_pool(name="qk", bufs=2))
        attp = es.enter_context(tc.tile_pool(name="att", bufs=2))
        tmp = es.enter_context(tc.tile_pool(name="tmp", bufs=2))
        small = es.enter_context(tc.tile_pool(name="small", bufs=2))
        obuf = es.enter_context(tc.tile_pool(name="ob", bufs=2))
        # lam = exp(sum(lq1*lk1)) - exp(sum(lq2*lk2)) + 0.8 ; store -lam
        l1 = cpool.tile([1, D], F32)
        l2 = cpool.tile([1, D], F32)
        nc.vector.tensor_tensor(l1[:], lam_t[0:1, 0:D], lam_t[0:1, D:2 * D],
                                op=ALU.mult)
        nc.vector.tensor_tensor(l2[:], lam_t[0:1, 2 * D:3 * D],
                                lam_t[0:1, 3 * D:4 * D], op=ALU.mult)
        s1 = cpool.tile([1, 1], F32)
        s2 = cpool.tile([1, 1], F32)
        nc.vector.reduce_sum(s1[:], l1[:], axis=mybir.AxisListType.X)
        nc.vector.reduce_sum(s2[:], l2[:], axis=mybir.AxisListType.X)
        e1s = cpool.tile([1, 1], F32)
        e2s = cpool.tile([1, 1], F32)
        nc.scalar.activation(e1s[:], s1[:], AF.Exp)
        nc.scalar.activation(e2s[:], s2[:], AF.Exp)
        dls = cpool.tile([1, 1], F32)
        nc.vector.tensor_tensor(dls[:], e1s[:], e2s[:], op=ALU.subtract)
        lamneg_t = cpool.tile([1, 1], F32)
        nc.scalar.activation(lamneg_t[:], dls[:], AF.Copy, scale=-1.0,
                             bias=-LAM_INIT)
        qf = big.tile([128, 8, NQ], F32R)       # [2 paths x 64 d, head, qrow]
        kf = big.tile([128, 8, R], F32R)

        def qk_proj(w_sb, xT, n, out_f, m_t):
            for h in range(8):
                psq = ps.tile([128, n], F32, tag="ps")
                for kc in range(8):
                    nc.tensor.matmul(psq[:], w_sb[kc][:, 128 * h:128 * h + 128],
                                     xT[:, kc, :], start=(kc == 0),
                                     stop=(kc == 7))
                rawf = qkp.tile([128, 512], F32R, tag="raw")
                raw = rawf[:, 0:n]
                nc.vector.tensor_copy(raw, psq[:])
                sqf = qkp.tile([128, 512], F32R, tag="sq")
                sq = sqf[:, 0:n]
                nc.vector.tensor_tensor(sq, raw.bitcast(F32),
                                        raw.bitcast(F32), op=ALU.mult)
                pss = ps.tile([33, n], F32, tag="ps")
                nc.tensor.matmul(pss[:], m2_t[:], sq, start=True, stop=True)
                repf = qkp.tile([128, 512], F32, tag="rep")
                rep = repf[:, 0:n]
                for p in range(2):
                    tqf = small.tile([1, 512], F32, tag="tq", name="tq%d" % p)
                    tq = tqf[:, 0:n]
                    nc.scalar.activation(tq, pss[32 * p:32 * p + 1, :], AF.Sqrt,
                                         bias=eps2_t[0:1, :], scale=1.0 / D)
                    rqf = small.tile([1, 512], F32, tag="rq", name="rq%d" % p)
                    rq = rqf[:, 0:n]
                    nc.vector.reciprocal_approx_fast(rq, tq)
                    nc.gpsimd.partition_broadcast(rep[64 * p:64 * p + 64, :], rq)
                for p in range(2):
                    pp = ps.tile([64, n], F32, tag="ps")
                    sl = slice(64 * p, 64 * p + 64)
                    nc.tensor.matmul(pp[:], m_t[sl, 64 * h:64 * h + 64],
                                     raw[sl, :], start=True, stop=True)
                    nc.vector.tensor_tensor(out_f[sl, h, :], pp[:],
                                            rep[sl, :], op=ALU.mult)

        wq_sb = []
        for kc in range(8):
            t = wpool.tile([128, C], F32R, tag="w")
            nc.sync.dma_start(t[:], i_wq[128 * kc:128 * kc + 128, :])
            wq_sb.append(t)
        qk_proj(wq_sb, xqT, NQ, qf, mq_t)
        wk_sb = []
        for kc in range(8):
            t = wpool.tile([128, C], F32R, tag="w")
            nc.sync.dma_start(t[:], i_wk[128 * kc:128 * kc + 128, :])
            wk_sb.append(t)
        qk_proj(wk_sb, xdT, R, kf, mk_t)

        # ---- V projection (row-major) ----
        v_t = big.tile([128, 4, C], F32R)       # [krow_chunk, rc, vch]
        wv_sb = []
        for kc in range(8):
            t = wpool.tile([128, C], F32R, tag="w")
            nc.sync.dma_start(t[:], i_wv[128 * kc:128 * kc + 128, :])
            wv_sb.append(t)
        for rc in range(4):
            for n2 in range(2):
                psv = ps.tile([128, 512], F32, tag="ps")
                for kc in range(8):
                    nc.tensor.matmul(psv[:], xdT[:, kc, 128 * rc:128 * rc + 128],
                                     wv_sb[kc][:, 512 * n2:512 * n2 + 512],
                                     start=(kc == 0), stop=(kc == 7))
                nc.scalar.copy(v_t[:, rc, 512 * n2:512 * n2 + 512], psv[:])

        # ---- attention + head norm, per head ----
        dn = big.tile([128, 8, NQ], F32R)       # [in_ch_chunk, h, qrow]
        for h in range(8):
            ex = [attp.tile([128, 4, NQ], F32R, tag="e%d" % p, name="ex%d" % p)
                  for p in range(2)]
            for p in range(2):
                sl = slice(64 * p, 64 * p + 64)
                for rc in range(4):
                    pssc = ps.tile([128, NQ], F32, tag="ps")
                    nc.tensor.matmul(pssc[:], kf[sl, h, 128 * rc:128 * rc + 128],
                                     qf[sl, h, :], start=True, stop=True)
                    nc.scalar.activation(ex[p][:, rc, :], pssc[:], AF.Exp)
            reps = []
            for p in range(2):
                pssum = ps.tile([1, NQ], F32, tag="ps")
                for rc in range(4):
                    nc.tensor.matmul(pssum[:], ones128_t[:], ex[p][:, rc, :],
                                     start=(rc == 0), stop=(rc == 3))
                rs = small.tile([1, NQ], F32, tag="rs")
                nc.vector.reciprocal_approx_fast(rs[:], pssum[:])
                if p == 1:
                    rs2 = small.tile([1, NQ], F32, tag="rs")
                    nc.vector.tensor_scalar_mul(rs2[:], rs[:], lamneg_t[:])
                    rs = rs2
                rep = tmp.tile([128, NQ], F32, tag="rep")
                nc.gpsimd.partition_broadcast(rep[:], rs[:])
                reps.append(rep)
            parts = []
            for p in range(2):
                pspv = ps.tile([128, NQ], F32, tag="ps")
                for rc in range(4):
                    nc.tensor.matmul(pspv[:], v_t[:, rc, 128 * h:128 * h + 128],
                                     ex[p][:, rc, :], start=(rc == 0),
                                     stop=(rc == 3))
                t = tmp.tile([128, NQ], F32, tag="pvt")
                nc.vector.tensor_tensor(t[:], pspv[:], reps[p][:], op=ALU.mult)
                parts.append(t)
            draw = tmp.tile([128, NQ], F32, tag="draw")
            nc.vector.tensor_tensor(draw[:], parts[0][:], parts[1][:],
                                    op=ALU.add)
            # head rmsnorm over 128 channels
            sqd = tmp.tile([128, NQ], F32R, tag="sqd")
            nc.vector.tensor_tensor(sqd[:], draw[:], draw[:], op=ALU.mult)
            pshs = ps.tile([1, NQ], F32, tag="ps")
            nc.tensor.matmul(pshs[:], ones128_t[:], sqd[:], start=True,
                             stop=True)
            th = small.tile([1, NQ], F32, tag="th")
            nc.scalar.activation(th[:], pshs[:], AF.Sqrt,
                                 bias=eps2_t[0:1, :], scale=1.0 / 128)
            rh = small.tile([1, NQ], F32, tag="rh")
            nc.vector.reciprocal_approx_fast(rh[:], th[:])
            reph = tmp.tile([128, NQ], F32, tag="rep")
            nc.gpsimd.partition_broadcast(reph[:], rh[:])
            dnt = tmp.tile([128, NQ], F32, tag="dnt")
            nc.vector.tensor_tensor(dnt[:], reph[:], draw[:], op=ALU.mult)
            nc.vector.tensor_scalar_mul(dn[:, h, :], dnt[:], hw2_t[:])

        # ---- Wo + silu ----
        lr = big.tile([128, 3, C], F32R)
        wo_sb = []
        for kc in range(8):
            t = wpool.tile([128, C], F32R, tag="w")
            nc.sync.dma_start(t[:], i_wo[128 * kc:128 * kc + 128, :])
            wo_sb.append(t)
        mrows = (128, 128, 2)
        for mc in range(3):
            rows = mrows[mc]
            for n2 in range(2):
                pso = ps.tile([128, 512], F32, tag="ps")
                for kc in range(8):
                    nc.tensor.matmul(pso[0:rows, :],
                                     dn[:, kc, 128 * mc:128 * mc + rows],
                                     wo_sb[kc][:, 512 * n2:512 * n2 + 512],
                                     start=(kc == 0), stop=(kc == 7))
                nc.scalar.activation(lr[0:rows, mc, 512 * n2:512 * n2 + 512],
                                     pso[0:rows, :], AF.Silu)

        # ---- upsample (banded interp matmuls) + outputs ----
        ub_t = {}
        ubb_t = cpool.tile([2, 16 * 128], F32R, name="ubbt")
        nc.sync.dma_start(ubb_t[:], i_ubb[:])
        for half in range(2):
            ta = wpool.tile([128, C], F32R, tag="w", name="uba%d" % half)
            nc.sync.dma_start(ta[:], i_uba[:, C * half:C * half + C])
            ub_t[half] = ta
        for oc in range(16):
            p0, crossing = segs[oc]
            uba_t = ub_t[oc // 8]
            col = 128 * (oc % 8)
            colg = 128 * oc
            ob = obuf.tile([128, C], F32, tag="ob")
            for n2 in range(2):
                psu = ps.tile([128, 512], F32, tag="ps")
                nc.tensor.matmul(psu[:], uba_t[:, col:col + 128],
                                 lr[:, p0, 512 * n2:512 * n2 + 512],
                                 start=True, stop=not crossing)
                if crossing:
                    nc.tensor.matmul(psu[:], ubb_t[0:2, colg:colg + 128],
                                     lr[0:2, p0 + 1, 512 * n2:512 * n2 + 512],
                                     start=False, stop=True)
                nc.any.tensor_copy(ob[:, 512 * n2:512 * n2 + 512], psu[:])
            nc.sync.dma_start(o_fo[128 * oc:128 * oc + 128, :], ob[:])
        # lowrank shard: local rows 1..256  ->  global [256*s, 256*s+256)
        nc.sync.dma_start(o_lo[0:127, :], lr[1:128, 0, :].bitcast(F32))
        nc.sync.dma_start(o_lo[127:255, :], lr[0:128, 1, :].bitcast(F32))
        nc.sync.dma_start(o_lo[255:256, :], lr[0:1, 2, :].bitcast(F32))
        es.close()
    nc.finalize()
    return nc


def _get_built():
    if "nc" not in _CACHE:
        _, _, segs = _upsample_tables(0)
        segs1 = _upsample_tables(1)[2]
        assert segs == segs1, "segment structure must be core-uniform"
        _CACHE["nc"] = _build_nc(segs)
    return _CACHE["nc"]


def _in_maps(x, Wq, Wk, Wv, Wo, q_norm_w, k_norm_w, head_norm_w,
             lambda_q1, lambda_k1, lambda_q2, lambda_k2):
    f = np.float32
    wqT = np.ascontiguousarray(Wq.T, dtype=f)
    wkT = np.ascontiguousarray(Wk.T, dtype=f)
    wvT = np.ascontiguousarray(Wv.T, dtype=f)
    woT = np.ascontiguousarray(Wo.T, dtype=f)
    rtq, rtk = _rt_tables()
    qw2 = np.tile(np.asarray(q_norm_w, f), 2).reshape(128, 1)
    kw2 = np.tile(np.asarray(k_norm_w, f), 2).reshape(128, 1)
    hw = np.asarray(head_norm_w, f).reshape(128, 1)
    lamvec = np.concatenate([np.asarray(a, f) for a in
                             (lambda_q1, lambda_k1, lambda_q2, lambda_k2)]
                            ).reshape(1, 4 * D)
    m2 = np.zeros((128, 33), f)
    m2[:D, 0] = 1.0
    m2[D:, 32] = 1.0
    e2 = np.zeros((2, 128), f)
    e2[0, :D] = 1.0
    e2[1, D:] = 1.0
    onesr = np.ones((1, 128), f)
    ones128 = np.ones((128, 1), f)
    ident = np.eye(128, dtype=f)
    shared = dict(wqT=wqT, wkT=wkT, wvT=wvT, woT=woT, rtq=rtq, rtk=rtk,
                  qw2=qw2, kw2=kw2, hw=hw, lamvec=lamvec, m2=m2, e2=e2,
                  onesr=onesr, ones128=ones128, ident=ident)
    ubs = [_upsample_tables(s)[:2] for s in range(2)]
    maps = []
    for c in range(NCORES):
        b, s = c // 2, c % 2
        xlo = np.ascontiguousarray(x[b, 3::8, :], dtype=f)
        xhi = np.ascontiguousarray(x[b, 4::8, :], dtype=f)
        A = 256 * s - 1
        xqlo = np.zeros((NQ, C), f)
        xqhi = np.zeros((NQ, C), f)
        g0, g1 = max(0, A), min(R, A + NQ)
        xqlo[g0 - A:g1 - A] = xlo[g0:g1]
        xqhi[g0 - A:g1 - A] = xhi[g0:g1]
        m = dict(shared)
        m.update(xlo=xlo, xhi=xhi, xqlo=xqlo, xqhi=xqhi,
                 ub_a=ubs[s][0], ub_b=ubs[s][1])
        maps.append(m)
    return maps


def kernel(**inputs):
    from concourse import bass_utils
    nc = _get_built()
    maps = _in_maps(**inputs)
    res = bass_utils.run_bass_kernel_spmd(nc, maps, core_ids=list(range(NCORES)))
    full = np.empty((B, L, C), np.float32)
    low = np.empty((B, R, C), np.float32)
    for c in range(NCORES):
        b, s = c // 2, c % 2
        full[b, LSH * s:LSH * s + LSH, :] = res.results[c]["fo"]
        low[b, 256 * s:256 * s + 256, :] = res.results[c]["lo"]
    return full, low
